# revision 16
# baseline (speedup 1.0000x reference)
"""BackpropWiSARD embedding-lookup kernel for 8 Trainium2 NeuronCores, v3.

Data-parallel over batch (64 rows/core), table replicated, bf16 classes.

Key idea vs v2: the hash pipeline computes gather indices DIRECTLY in
dma_gather's wrap layout, eliminating the DRAM index-shuffle round trip
(which dominated the baseline at ~900us of 2-byte scattered descriptors).

dma_gather ucode facts used:
  - call on queue q is processed by Q7 cpu pair (2q, 2q+1), which reads the
    index tile ONLY from partitions [32q, 32q+32) (two 16-row replicas).
  - wrap layout: index j of a call sits at [j%16, j//16] of those rows.

Layouts (per core):
  - partitions p = q*32 + r*16 + b_lo  (q = queue, r = replica duplicate,
    b_lo = b%16).  Window g = gq*4 + q holds f's {fl*56+g} (fl = flh*2+fl2),
    rows fl*8192+e of the 32768-row window; gathers for window g are issued
    on queue q, so partitions of queue q hold exactly its windows' indices.
  - mapped[p, gq, 1, (flh fl2 b_hi), i] int16 = x[b, input_order[f*28+i]]
    (host applies the input_order permutation; pure data movement).
  - hash: msk = mapped * hv[h, i] (broadcast over h), XOR-tree over i=28,
    XOR with fl*8192 -> written to idxT[:, gq*64 + h*16 + flh*8 + fl2*4 + b_hi].
  - call (gq, q): idxs_ap = idxT[:, gq*64:(gq+1)*64] for every q; queue q's
    cpu pair sees its own windows' indices.  j = col*16 + b_lo =>
    Mc partition p_dst = fl2*64 + b, slot j//128 = h*2 + flh.
  - reduce: min over h (slot pairs), binarize (is_ge 0), add flh pair, sum
    windows, accumulate; PE matmul with a (128,64) selection matrix folds the
    fl2 partition pairs; affine 2S-F + bias; out (64,100) f32 per core.
"""

import sys

sys.path.insert(0, "/opt/trn_rl_repo")

import numpy as np
import ml_dtypes

B, C, F, E, H, I = 512, 100, 224, 8192, 4, 28
NB = F * I
NCORES = 8
BP = B // NCORES  # 64
CP = 128
GW = 4 * E  # 32768-row window
NG = 56  # windows; window g holds f = fl*56 + g
NGQ = 14  # window-column groups (g = gq*4 + q)
GRP = 16  # (flh, fl2, b_hi)
CHUNK = 2  # gq per chunk
NCHUNK = NGQ // CHUNK  # 7

_NC = None


def _build(loop_reps=1):
    import contextlib

    import concourse.bass as bass
    import concourse.mybir as mybir
    import concourse.tile as tile
    from concourse import bacc
    from concourse.library_config import mlp

    dt = mybir.dt
    op = mybir.AluOpType

    nc = bacc.Bacc(
        "TRN2", target_bir_lowering=False, debug=False, num_swdge_queues=4
    )

    tbl = nc.dram_tensor("tbl", (NG * GW, CP), dt.bfloat16, kind="ExternalInput")
    mapd = nc.dram_tensor("mapd", (128, NGQ * GRP * I), dt.int16, kind="ExternalInput")
    hvx = nc.dram_tensor("hvx", (128, H * GRP * I), dt.int16, kind="ExternalInput")
    rofs = nc.dram_tensor("rofs", (128, GRP), dt.int16, kind="ExternalInput")
    sel = nc.dram_tensor("sel", (CP, BP), dt.bfloat16, kind="ExternalInput")
    biasx = nc.dram_tensor("biasx", (BP, C), dt.float32, kind="ExternalInput")
    outd = nc.dram_tensor("out", (BP, C), dt.float32, kind="ExternalOutput")

    with tile.TileContext(nc) as tc:
        nc.gpsimd.load_library(mlp)
        with (
            tc.tile_pool(name="main", bufs=1) as pool,
            tc.tile_pool(name="msk", bufs=2) as kpool,
            tc.tile_pool(name="mc", bufs=4) as mpool,
            tc.tile_pool(name="psum", bufs=2, space="PSUM") as psum_pool,
            (tc.For_i(0, loop_reps, 1) if loop_reps > 1 else contextlib.nullcontext()),
        ):
            # [p, gq, 1(h), grp, i]
            mapped = pool.tile([128, NGQ, 1, GRP, I], dt.int16)
            nc.sync.dma_start(
                out=mapped[:].rearrange("p gq o grp i -> p (gq o grp i)"),
                in_=mapd.ap(),
            )
            hvp = pool.tile([128, 1, H, GRP * I], dt.int16)
            nc.sync.dma_start(
                out=hvp[:].rearrange("p o h gi -> p (o h gi)"), in_=hvx.ap()
            )
            rofs_sb = pool.tile([128, 1, 1, GRP], dt.int16)
            nc.sync.dma_start(
                out=rofs_sb[:].rearrange("p o z grp -> p (o z grp)"), in_=rofs.ap()
            )
            bias_sb = pool.tile([BP, C], dt.float32)
            nc.sync.dma_start(out=bias_sb[:], in_=biasx.ap())
            sel_sb = pool.tile([CP, BP], dt.bfloat16)
            nc.sync.dma_start(out=sel_sb[:], in_=sel.ap())

            idxT = pool.tile([128, NGQ, H, GRP], dt.int16)
            acc = pool.tile([CP, 1, 1, CP], dt.bfloat16)
            nc.vector.memset(acc[:], 0)

            mcs = []

            def reduce_mc(Mc):
                # slots s = h*2 + flh; min over h, binarize, sum slabs
                nc.vector.tensor_tensor(
                    out=Mc[:, :, 0:4, :],
                    in0=Mc[:, :, 0:4, :],
                    in1=Mc[:, :, 4:8, :],
                    op=op.min,
                )
                nc.vector.tensor_tensor(
                    out=Mc[:, :, 0:2, :],
                    in0=Mc[:, :, 0:2, :],
                    in1=Mc[:, :, 2:4, :],
                    op=op.min,
                )
                nc.vector.tensor_scalar(
                    out=Mc[:, :, 0:2, :],
                    in0=Mc[:, :, 0:2, :],
                    scalar1=0.0,
                    scalar2=None,
                    op0=op.is_ge,
                )
                nc.vector.tensor_tensor(
                    out=Mc[:, :, 0:1, :],
                    in0=Mc[:, :, 0:1, :],
                    in1=Mc[:, :, 1:2, :],
                    op=op.add,
                )
                w = CHUNK * 4
                while w > 1:
                    lo = w // 2
                    nc.vector.tensor_tensor(
                        out=Mc[:, 0:lo, 0:1, :],
                        in0=Mc[:, 0:lo, 0:1, :],
                        in1=Mc[:, lo : 2 * lo, 0:1, :],
                        op=op.add,
                    )
                    w = lo
                nc.vector.tensor_tensor(
                    out=acc[:],
                    in0=acc[:],
                    in1=Mc[:, 0:1, 0:1, :],
                    op=op.add,
                )

            for c in range(NCHUNK):
                gqs = slice(c * CHUNK, (c + 1) * CHUNK)
                msk = kpool.tile([128, CHUNK, H, GRP, I], dt.int16, tag="msk")
                nc.vector.tensor_tensor(
                    out=msk[:].rearrange("p gq h grp i -> p gq h (grp i)"),
                    in0=mapped[:, gqs, :, :, :]
                    .rearrange("p gq o grp i -> p gq o (grp i)")
                    .to_broadcast([128, CHUNK, H, GRP * I]),
                    in1=hvp[:].to_broadcast([128, CHUNK, H, GRP * I]),
                    op=op.mult,
                )
                # XOR-reduce over i=28: 14/7/(3,3)+tail
                msk2 = msk[:].rearrange("p gq h grp i -> p (gq h grp) i")
                for (d0, w) in ((14, 14), (7, 7), (3, 3), (1, 1), (2, 1), (6, 1)):
                    nc.vector.tensor_tensor(
                        out=msk2[:, :, 0 : min(w, d0)],
                        in0=msk2[:, :, 0 : min(w, d0)],
                        in1=msk2[:, :, d0 : d0 + w],
                        op=op.bitwise_xor,
                    )
                # idx = msk ^ fl*8192 -> wrap columns
                nc.vector.tensor_tensor(
                    out=idxT[:, gqs, :, :],
                    in0=msk[:, :, :, :, 0:1].rearrange("p gq h grp o -> p gq h (grp o)"),
                    in1=rofs_sb[:].to_broadcast([128, CHUNK, H, GRP]),
                    op=op.bitwise_xor,
                )

                Mc = mpool.tile([128, CHUNK * 4, 2 * H, CP], dt.bfloat16, tag="Mc")
                mcs.append(Mc)
                for gql in range(CHUNK):
                    gq = c * CHUNK + gql
                    for q in range(4):
                        g = gq * 4 + q
                        nc.gpsimd.dma_gather(
                            out_ap=Mc[:, gql * 4 + q, :, :],
                            in_ap=tbl.ap()[g * GW : (g + 1) * GW, :],
                            idxs_ap=idxT[:, gq, :, :].rearrange(
                                "p h grp -> p (h grp)"
                            ),
                            num_idxs=1024,
                            num_idxs_reg=1024,
                            elem_size=CP,
                            queue_num=q,
                        )
                if c >= 1:
                    reduce_mc(mcs[c - 1])
            reduce_mc(mcs[NCHUNK - 1])

            S = psum_pool.tile([BP, CP], dt.float32, tag="S")
            nc.tensor.matmul(
                out=S[:],
                lhsT=sel_sb[:],
                rhs=acc[:].rearrange("p o z c -> p (o z c)"),
                start=True,
                stop=True,
            )
            res = pool.tile([BP, C], dt.float32)
            nc.vector.tensor_scalar(
                out=res[:],
                in0=S[:, 0:C],
                scalar1=2.0,
                scalar2=float(-F),
                op0=op.mult,
                op1=op.add,
            )
            nc.vector.tensor_tensor(out=res[:], in0=res[:], in1=bias_sb[:], op=op.add)
            nc.sync.dma_start(out=outd.ap(), in_=res[:])

    nc.compile()
    return nc


def get_nc(loop_reps=1):
    global _NC
    if loop_reps != 1:
        return _build(loop_reps)
    if _NC is None:
        _NC = _build()
    return _NC


def prep_in_maps(inputs):
    x_b = np.asarray(inputs["x_b"], dtype=np.int32)
    input_order = np.asarray(inputs["input_order"], dtype=np.int32)
    hash_values = np.asarray(inputs["hash_values"], dtype=np.int32)
    table = np.asarray(inputs["table"], dtype=np.float32)
    bias = np.asarray(inputs["bias"], dtype=np.float32)

    # (C,F,E) -> rows [(g*4+fl)*8192 + e] = f = fl*56 + g, 128-class bf16 rows
    tp = np.zeros((F, E, CP), dtype=ml_dtypes.bfloat16)
    tp[:, :, :C] = table.transpose(1, 2, 0).astype(ml_dtypes.bfloat16)
    g_ = np.arange(NG)[:, None]
    fl_ = np.arange(4)[None, :]
    fmap = (fl_ * NG + g_).reshape(-1)  # (g,fl) -> f
    tt = np.ascontiguousarray(tp[fmap]).reshape(NG * GW, CP)

    # per-core mapped[p=(q,r,b_lo), gq, grp=(flh,fl2,b_hi), i] =
    #   x[b_hi*16+b_lo, input_order[f*28+i]],  f=(flh*2+fl2)*56 + gq*4 + q
    q_ = np.arange(4)[:, None, None, None, None, None]  # q
    gq_ = np.arange(NGQ)[None, None, None, :, None, None]
    flh_ = np.arange(2)[None, None, None, None, :, None]
    fl2_ = np.arange(2)[None, None, None, None, None, :]
    f_full = (flh_ * 2 + fl2_) * NG + gq_ * 4 + q_  # [4,1,1,14,2,2]
    i_ = np.arange(I)
    bit_idx = f_full[..., None] * I + i_  # [4,1,1,14,2,2,28]
    bit_idx = np.broadcast_to(bit_idx, (4, 2, 16, NGQ, 2, 2, I))
    order_bits = input_order[bit_idx]  # bit position per slot

    hvx = np.ascontiguousarray(
        np.broadcast_to(
            hash_values[None, :, None, :].astype(np.int16), (128, H, GRP, I)
        )
    ).reshape(128, H * GRP * I)

    grp_ = np.arange(GRP)
    rofs = np.ascontiguousarray(
        np.broadcast_to(((grp_ // 4) * E).astype(np.int16)[None, :], (128, GRP))
    )

    selm = np.ascontiguousarray(
        np.tile(np.eye(BP, dtype=np.float32), (2, 1)).astype(ml_dtypes.bfloat16)
    )
    biasx = np.ascontiguousarray(np.tile(bias.reshape(1, C), (BP, 1)))

    b_lo_ = np.arange(16)[None, None, :, None, None, None]
    b_hi_ = np.arange(4)
    in_maps = []
    for k in range(NCORES):
        xk = x_b[k * BP : (k + 1) * BP]  # (64, 6272)
        b_full = (b_hi_[None, None, None, None, None, None, :] * 16 + b_lo_[..., None])
        # shape [4(q),2(r),16(b_lo),14,2,2,4(b_hi)] -> batch index
        b_full = np.broadcast_to(b_full, (4, 2, 16, NGQ, 2, 2, 4))
        # mapped value = xk[b, order_bits[...]] with grp=(flh,fl2,b_hi)
        ob = np.broadcast_to(
            order_bits[:, :, :, :, :, :, None, :], (4, 2, 16, NGQ, 2, 2, 4, I)
        )
        bf = np.broadcast_to(b_full[..., None], (4, 2, 16, NGQ, 2, 2, 4, I))
        mapd = xk[bf, ob].astype(np.int16)  # [q,r,b_lo,gq,flh,fl2,b_hi,i]
        mapd = np.ascontiguousarray(mapd.reshape(128, NGQ * GRP * I))
        in_maps.append(
            {
                "tbl": tt,
                "mapd": mapd,
                "hvx": hvx,
                "rofs": rofs,
                "sel": selm,
                "biasx": biasx,
            }
        )
    return in_maps


def kernel(**inputs):
    from concourse.bass_utils import run_bass_kernel_spmd

    nc = get_nc()
    in_maps = prep_in_maps(inputs)
    res = run_bass_kernel_spmd(nc, in_maps, list(range(NCORES)))
    parts = [res.results[k]["out"].reshape(BP, C) for k in range(NCORES)]
    return np.concatenate(parts, axis=0).astype(np.float32)
